# revision 9
# baseline (speedup 1.0000x reference)
"""Trainium2 Bass kernel for nn_CPA_43 (dense transformer block, CPA attention).

Data-parallel over batch: B=256 sharded as 32 samples per core across 8 cores.
All weights replicated. Two on-chip stages per core:
  stage 1: LN1/LN2, Q/K/V projections, channel-softmax(q), position-softmax(k),
           context/attention matmuls, Wr + residual -> f3out (spilled to DRAM),
           LN3 statistics on the fly.
  stage 2: LN3 apply, MLP (W1 -> gelu -> W2) + residual -> output.

Matmuls run in float32r (full PE rate for free dim >= 256, ~1.5e-4 rel err).
Per-channel / per-position biases are preloaded into PSUM with identity/ones
matmuls; LN gains and positional-encoding projections are folded on the host.
"""

import numpy as np

B, N3, N4, DIM, HEADS, MLP_DIM = 256, 256, 64, 512, 8, 2048
N_CORES = 8
BSH = B // N_CORES  # samples per core
EPS = 1e-5

_BUILD_CACHE = {}


def _host_prep(inputs):
    """Fold LN gains + positional projections into weights/biases (exact)."""
    f = {k: np.asarray(v, dtype=np.float64) for k, v in inputs.items()}
    pos3 = f["pos3"][0]  # [N3, DIM]
    pos4 = f["pos4"][0]  # [N4, DIM]

    wq = np.ascontiguousarray((f["ln1_g"][:, None] * f["Wq"]).astype(np.float32))
    wk = np.ascontiguousarray((f["ln2_g"][:, None] * f["Wk"]).astype(np.float32))
    wv = np.ascontiguousarray((f["ln2_g"][:, None] * f["Wv"]).astype(np.float32))
    wr = np.ascontiguousarray(f["Wr"].astype(np.float32))
    w1 = np.ascontiguousarray((f["ln3_g"][:, None] * f["W1"]).astype(np.float32))
    w2 = np.ascontiguousarray(f["W2"].astype(np.float32))

    biasq = ((f["ln1_b"][None, :] + pos3) @ f["Wq"] + f["bq"]).astype(np.float32)  # [N3, DIM]
    biask = ((f["ln2_b"][None, :] + pos4) @ f["Wk"] + f["bk"]).astype(np.float32)  # [N4, DIM]
    biasv = (f["ln2_b"] @ f["Wv"] + f["bv"]).astype(np.float32)  # [DIM]
    bias1 = (f["ln3_b"] @ f["W1"] + f["b1"]).astype(np.float32)  # [MLP]
    br = f["br"].astype(np.float32)
    b2 = f["b2"].astype(np.float32)

    # biask in channel-major, tiled over the 4 samples of an f4-group:
    # [DIM, 4*N4] with column order (sample_in_group, position)
    biask_cm = np.tile(biask.T[:, None, :], (1, 4, 1)).reshape(DIM, 4 * N4)
    biask_cm = np.ascontiguousarray(biask_cm.astype(np.float32))
    # bias1 as [128, 16]: column hc holds biases for hidden channels hc*128..+128
    bias1_cm = np.ascontiguousarray(bias1.reshape(MLP_DIM // 128, 128).T.astype(np.float32))

    return {
        "wq": wq, "wk": wk, "wv": wv, "wr": wr, "w1": w1, "w2": w2,
        "biasq": np.ascontiguousarray(biasq),
        "biask_cm": biask_cm,
        "biasv_row": np.ascontiguousarray(biasv[None, :]),
        "br_row": np.ascontiguousarray(br[None, :]),
        "b2_row": np.ascontiguousarray(b2[None, :]),
        "bias1_cm": bias1_cm,
        "ones_col": np.ones((1, 128), dtype=np.float32),
        "ident": np.eye(128, dtype=np.float32),
    }


def _build(n_samples):
    """Build the Bacc module for one core processing `n_samples` samples."""
    import concourse.bacc as bacc
    import concourse.tile as tile
    import concourse.mybir as mybir
    from concourse.bass import AP  # noqa: F401

    # Restrict activation-table-set choices so stage 1 (Exp+Ln+Square+Copy)
    # stays on natural_log_exp_and_others and stage 2 (Gelu+Square+Copy) on
    # gelu_and_others — otherwise the placement pass alternates between the
    # exp-only and ln-only sets, paying ~2.7us per reload, ~70 times.
    import concourse.hw_specs as hw_specs
    if not hasattr(bacc, "_orig_get_activation_tables"):
        bacc._orig_get_activation_tables = bacc.get_activation_tables

        def _gat(arch):
            full = bacc._orig_get_activation_tables(arch)
            keep = {"natural_log_exp_and_others", "gelu_and_others"}
            return {n: (s if n in keep else set()) for n, s in full.items()}

        bacc.get_activation_tables = _gat

    F32 = mybir.dt.float32
    F32R = mybir.dt.float32r
    AX = mybir.AxisListType.X
    ALU = mybir.AluOpType
    ACTF = mybir.ActivationFunctionType

    NS = n_samples
    assert NS % 4 == 0
    NG4 = NS // 4        # f4 groups of 4 samples
    NG2 = NS // 2        # mlp groups of 2 samples

    nc = bacc.Bacc("TRN2", debug=False, num_devices=N_CORES)

    f3 = nc.dram_tensor("f3", [NS, N3, DIM], F32, kind="ExternalInput").ap()
    f4 = nc.dram_tensor("f4", [NS, N4, DIM], F32, kind="ExternalInput").ap()
    wq = nc.dram_tensor("wq", [DIM, DIM], F32R, kind="ExternalInput").ap()
    wk = nc.dram_tensor("wk", [DIM, DIM], F32R, kind="ExternalInput").ap()
    wv = nc.dram_tensor("wv", [DIM, DIM], F32R, kind="ExternalInput").ap()
    wr = nc.dram_tensor("wr", [DIM, DIM], F32R, kind="ExternalInput").ap()
    w1 = nc.dram_tensor("w1", [DIM, MLP_DIM], F32R, kind="ExternalInput").ap()
    w2 = nc.dram_tensor("w2", [MLP_DIM, DIM], F32R, kind="ExternalInput").ap()
    biasq = nc.dram_tensor("biasq", [N3, DIM], F32R, kind="ExternalInput").ap()
    biask_cm = nc.dram_tensor("biask_cm", [DIM, 4 * N4], F32R, kind="ExternalInput").ap()
    biasv_row = nc.dram_tensor("biasv_row", [1, DIM], F32R, kind="ExternalInput").ap()
    br_row = nc.dram_tensor("br_row", [1, DIM], F32R, kind="ExternalInput").ap()
    b2_row = nc.dram_tensor("b2_row", [1, DIM], F32R, kind="ExternalInput").ap()
    bias1_cm = nc.dram_tensor("bias1_cm", [128, MLP_DIM // 128], F32, kind="ExternalInput").ap()
    ones_col = nc.dram_tensor("ones_col", [1, 128], F32R, kind="ExternalInput").ap()
    ident = nc.dram_tensor("ident", [128, 128], F32R, kind="ExternalInput").ap()
    out = nc.dram_tensor("out", [NS, N3, DIM], F32, kind="ExternalOutput").ap()

    with tile.TileContext(nc) as tc:
        # ---- pools alive for the whole kernel ----
        with (
            tc.tile_pool(name="consts", bufs=1) as cpool,
            tc.tile_pool(name="wattn", bufs=1) as wpool,
            tc.tile_pool(name="stats", bufs=1) as spool,
            tc.tile_pool(name="dram", bufs=1, space="DRAM") as dpool,
        ):
            ident_sb = cpool.tile([128, 128], F32R, tag="ident")
            nc.sync.dma_start(ident_sb[:], ident)
            eps_sb = cpool.tile([128, 1], F32, tag="eps")
            nc.vector.memset(eps_sb[:], EPS)
            ones_sb = cpool.tile([1, 128], F32R, tag="ones")
            nc.sync.dma_start(ones_sb[:], ones_col)
            bvrow_sb = cpool.tile([1, DIM], F32R, tag="bvrow")
            nc.sync.dma_start(bvrow_sb[:], biasv_row)
            brrow_sb = cpool.tile([1, DIM], F32R, tag="brrow")
            nc.sync.dma_start(brrow_sb[:], br_row)
            b2row_sb = cpool.tile([1, DIM], F32R, tag="b2row")
            nc.sync.dma_start(b2row_sb[:], b2_row)
            bq_sb = cpool.tile([128, 2, DIM], F32R, tag="bq")
            nc.sync.dma_start(bq_sb[:], biasq.rearrange("(t p) d -> p t d", p=128))
            bk_sb = cpool.tile([128, 4, 4 * N4], F32R, tag="bk")
            nc.sync.dma_start(bk_sb[:], biask_cm.rearrange("(c p) d -> p c d", p=128))
            b1_sb = cpool.tile([128, MLP_DIM // 128], F32, tag="b1")
            nc.sync.dma_start(b1_sb[:], bias1_cm)
            # persistent block-diagonal context tiles (off-diagonal stays zero)
            ctxbd = cpool.tile([128, 4, 128], F32R, tag="ctxbd")
            nc.vector.memset(ctxbd.bitcast(F32)[:], 0.0)

            wq_sb = wpool.tile([128, 4, DIM], F32R, tag="wq")
            nc.sync.dma_start(wq_sb[:], wq.rearrange("(c p) d -> p c d", p=128))
            wk_sb = wpool.tile([128, 4, DIM], F32R, tag="wk")
            nc.sync.dma_start(wk_sb[:], wk.rearrange("(c p) d -> p c d", p=128))
            wv_sb = wpool.tile([128, 4, DIM], F32R, tag="wv")
            nc.sync.dma_start(wv_sb[:], wv.rearrange("(c p) d -> p c d", p=128))
            wr_sb = wpool.tile([128, 4, DIM], F32R, tag="wr")
            nc.sync.dma_start(wr_sb[:], wr.rearrange("(c p) d -> p c d", p=128))

            # LN3 stats accumulated during stage 1 (per token-chunk column)
            sum3 = spool.tile([128, 2 * NS], F32, tag="sum3")
            sq3 = spool.tile([128, 2 * NS], F32, tag="sq3")
            negm3 = spool.tile([128, 2 * NS], F32, tag="negm3")
            s3 = spool.tile([128, 2 * NS], F32, tag="s3")

            f3o_dram = dpool.tile([NS, N3, DIM], F32, tag="f3spill")

            # ================= STAGE 1 =================
            with (
                tc.tile_pool(name="s1_sb", bufs=2) as p1,
                tc.tile_pool(name="s1_sb3", bufs=3) as p13,
                tc.tile_pool(name="kv", bufs=2) as pkv,
                tc.tile_pool(name="ps_tp", bufs=2, space="PSUM") as ps_tp,
                tc.tile_pool(name="ps_mm", bufs=2, space="PSUM") as ps_mm,
                tc.tile_pool(name="ps_ctx", bufs=2, space="PSUM") as ps_ctx,
                tc.tile_pool(name="ps_att", bufs=2, space="PSUM") as ps_att,
            ):
                def ln_stats_math(sums, sqs, negm, sinv, sl):
                    """Per-token -mean and 1/std from sum and sum-of-squares."""
                    mean = p1.tile([128, sl.stop - sl.start], F32, tag="ln_mean")
                    nc.vector.tensor_scalar_mul(mean[:], sums[:, sl], 1.0 / DIM)
                    m2 = p1.tile([128, sl.stop - sl.start], F32, tag="ln_m2")
                    nc.vector.tensor_mul(m2[:], mean[:], mean[:])
                    var = p1.tile([128, sl.stop - sl.start], F32, tag="ln_var")
                    nc.vector.scalar_tensor_tensor(
                        var[:], sqs[:, sl], 1.0 / DIM, m2[:],
                        op0=ALU.mult, op1=ALU.subtract,
                    )
                    lnv = p1.tile([128, sl.stop - sl.start], F32, tag="ln_lnv")
                    nc.scalar.activation(lnv[:], var[:], ACTF.Ln, bias=eps_sb[:])
                    nc.scalar.activation(sinv[:, sl], lnv[:], ACTF.Exp, scale=-0.5)
                    nc.vector.tensor_scalar_mul(negm[:, sl], mean[:], -1.0)

                for g in range(NG4):
                    # ---------- f4 block: 4 samples, 256 tokens ----------
                    x4 = p1.tile([128, 2, DIM], F32, tag="x4")
                    for t in range(2):
                        nc.sync.dma_start(
                            x4[:, t, :],
                            f4[4 * g + 2 * t: 4 * g + 2 * t + 2].rearrange("a b d -> (a b) d"),
                        )
                    s4sum = p1.tile([128, 2], F32, tag="s4sum")
                    s4sq = p1.tile([128, 2], F32, tag="s4sq")
                    scr4 = p1.tile([128, DIM], F32, tag="scr4")
                    for t in range(2):
                        nc.vector.reduce_sum(s4sum[:, t: t + 1], x4[:, t, :], axis=AX)
                        nc.scalar.activation(
                            scr4[:], x4[:, t, :], ACTF.Square, accum_out=s4sq[:, t: t + 1]
                        )
                    negm4 = p1.tile([128, 2], F32, tag="negm4")
                    sinv4 = p1.tile([128, 2], F32, tag="sinv4")
                    ln_stats_math(s4sum, s4sq, negm4, sinv4, slice(0, 2))
                    x4h = p1.tile([128, 2, DIM], F32R, tag="x4h")
                    for t in range(2):
                        nc.vector.tensor_scalar(
                            x4h[:, t, :], x4[:, t, :],
                            negm4[:, t: t + 1], sinv4[:, t: t + 1],
                            op0=ALU.add, op1=ALU.mult,
                        )
                    # transpose to channel-major [512ch, 256tok]
                    x4c = p1.tile([128, 4, 256], F32R, tag="x4c")
                    for cc in range(4):
                        ptp = ps_tp.tile([128, 512], F32R, tag="tp")
                        for t in range(2):
                            nc.tensor.transpose(
                                ptp[:, t * 128:(t + 1) * 128],
                                x4h[:, t, cc * 128:(cc + 1) * 128],
                                ident_sb[:],
                            )
                        nc.vector.tensor_copy(x4c[:, cc, :], ptp[:, 0:256])
                    # K projection (channel-major out) + bias preload + exp
                    kx = pkv.tile([128, 4, 256], F32R, tag="kx")
                    ks = p1.tile([128, 16], F32, tag="ks")
                    for cc in range(4):
                        pk = ps_mm.tile([128, 512], F32, tag="mm")
                        nc.tensor.matmul(
                            pk[:, 0:256], ident_sb[:], bk_sb[:, cc, :],
                            start=True, stop=False,
                        )
                        for kc in range(4):
                            nc.tensor.matmul(
                                pk[:, 0:256],
                                wk_sb[:, kc, cc * 128:(cc + 1) * 128],
                                x4c[:, kc, :],
                                start=False, stop=(kc == 3),
                            )
                        nc.scalar.activation(kx[:, cc, :], pk[:, 0:256], ACTF.Exp)
                        nc.vector.reduce_sum(
                            ks[:, cc * 4:(cc + 1) * 4],
                            kx[:, cc, :].rearrange("p (s d) -> p s d", s=4),
                            axis=AX,
                        )
                    kr = pkv.tile([128, 16], F32, tag="kr")
                    nc.vector.reciprocal(kr[:], ks[:])
                    # V projection (token-major out) + bias preload
                    v_tm = pkv.tile([128, 2, DIM], F32R, tag="v_tm")
                    for t in range(2):
                        pv = ps_mm.tile([128, 512], F32, tag="mm")
                        nc.tensor.matmul(pv[:], ones_sb[:], bvrow_sb[:], start=True, stop=False)
                        for kc in range(4):
                            nc.tensor.matmul(
                                pv[:],
                                x4c[:, kc, t * 128:(t + 1) * 128],
                                wv_sb[:, kc, :],
                                start=False, stop=(kc == 3),
                            )
                        nc.scalar.activation(v_tm[:, t, :], pv[:], ACTF.Copy)
                    # k back to token-major
                    k_tm = pkv.tile([128, 2, DIM], F32R, tag="k_tm")
                    for t in range(2):
                        ptp2 = ps_tp.tile([128, 512], F32R, tag="tp")
                        for cc in range(4):
                            nc.tensor.transpose(
                                ptp2[:, cc * 128:(cc + 1) * 128],
                                kx[:, cc, t * 128:(t + 1) * 128],
                                ident_sb[:],
                            )
                        nc.vector.tensor_copy(k_tm[:, t, :], ptp2[:])

                    # ---------- f3 blocks: 4 samples ----------
                    for si in range(4):
                        s = 4 * g + si
                        x3 = p13.tile([128, 2, DIM], F32, tag="x3")
                        for t in range(2):
                            nc.sync.dma_start(
                                x3[:, t, :], f3[s, t * 128:(t + 1) * 128, :]
                            )
                        s1sum = p1.tile([128, 2], F32, tag="s1sum")
                        s1sq = p1.tile([128, 2], F32, tag="s1sq")
                        scr3 = p1.tile([128, DIM], F32, tag="scr3")
                        for t in range(2):
                            nc.vector.reduce_sum(s1sum[:, t: t + 1], x3[:, t, :], axis=AX)
                            nc.scalar.activation(
                                scr3[:], x3[:, t, :], ACTF.Square,
                                accum_out=s1sq[:, t: t + 1],
                            )
                        negm1 = p1.tile([128, 2], F32, tag="negm1")
                        sinv1 = p1.tile([128, 2], F32, tag="sinv1")
                        ln_stats_math(s1sum, s1sq, negm1, sinv1, slice(0, 2))
                        x3h = p1.tile([128, 2, DIM], F32R, tag="x3h")
                        for t in range(2):
                            nc.vector.tensor_scalar(
                                x3h[:, t, :], x3[:, t, :],
                                negm1[:, t: t + 1], sinv1[:, t: t + 1],
                                op0=ALU.add, op1=ALU.mult,
                            )
                        x3c = p1.tile([128, 4, 256], F32R, tag="x3c")
                        for cc in range(4):
                            ptp = ps_tp.tile([128, 512], F32R, tag="tp")
                            for t in range(2):
                                nc.tensor.transpose(
                                    ptp[:, t * 128:(t + 1) * 128],
                                    x3h[:, t, cc * 128:(cc + 1) * 128],
                                    ident_sb[:],
                                )
                            nc.vector.tensor_copy(x3c[:, cc, :], ptp[:, 0:256])
                        # Q projection (token-major out) + biasq preload + exp
                        e_tm = p1.tile([128, 2, DIM], F32R, tag="e_tm")
                        qs = p1.tile([128, 16], F32, tag="qs")
                        for t in range(2):
                            pq = ps_mm.tile([128, 512], F32, tag="mm")
                            nc.tensor.matmul(
                                pq[:], ident_sb[:], bq_sb[:, t, :], start=True, stop=False
                            )
                            for kc in range(4):
                                nc.tensor.matmul(
                                    pq[:],
                                    x3c[:, kc, t * 128:(t + 1) * 128],
                                    wq_sb[:, kc, :],
                                    start=False, stop=(kc == 3),
                                )
                            nc.scalar.activation(e_tm[:, t, :], pq[:], ACTF.Exp)
                            nc.vector.reduce_sum(
                                qs[:, t * 8:(t + 1) * 8],
                                e_tm[:, t, :].rearrange("p (h d) -> p h d", h=8),
                                axis=AX,
                            )
                        qr = p1.tile([128, 16], F32, tag="qr")
                        nc.vector.reciprocal(qr[:], qs[:])
                        q_tm = p1.tile([128, 2, DIM], F32R, tag="q_tm")
                        for t in range(2):
                            nc.vector.tensor_tensor(
                                q_tm[:, t, :].rearrange("p (h d) -> p h d", h=8),
                                e_tm[:, t, :].rearrange("p (h d) -> p h d", h=8),
                                qr[:, t * 8:(t + 1) * 8].unsqueeze(-1).broadcast_to([128, 8, 64]),
                                op=ALU.mult,
                            )
                        q_cm = p1.tile([128, 4, 256], F32R, tag="q_cm")
                        for cc in range(4):
                            ptp = ps_tp.tile([128, 512], F32R, tag="tp")
                            for t in range(2):
                                nc.tensor.transpose(
                                    ptp[:, t * 128:(t + 1) * 128],
                                    q_tm[:, t, cc * 128:(cc + 1) * 128],
                                    ident_sb[:],
                                )
                            nc.vector.tensor_copy(q_cm[:, cc, :], ptp[:, 0:256])
                        # attention per head-pair
                        tb = si // 2
                        pb = (si % 2) * 64
                        att_cm = p1.tile([128, 4, 256], F32R, tag="att_cm")
                        for hp in range(4):
                            # ctx for both heads of the pair in one matmul;
                            # off-diagonal blocks are cross-head garbage.
                            pctx = ps_ctx.tile([128, 128], F32, tag="ctx")
                            nc.tensor.matmul(
                                pctx[:],
                                k_tm[pb:pb + 64, tb, hp * 128:(hp + 1) * 128],
                                v_tm[pb:pb + 64, tb, hp * 128:(hp + 1) * 128],
                                start=True, stop=True,
                            )
                            for hh in range(2):
                                nc.vector.tensor_scalar_mul(
                                    ctxbd[hh * 64:(hh + 1) * 64, hp, hh * 64:(hh + 1) * 64],
                                    pctx[hh * 64:(hh + 1) * 64, hh * 64:(hh + 1) * 64],
                                    kr[hh * 64:(hh + 1) * 64, hp * 4 + si: hp * 4 + si + 1],
                                )
                            patt = ps_att.tile([128, 256], F32, tag="att")
                            nc.tensor.matmul(
                                patt[:], ctxbd[:, hp, :], q_cm[:, hp, :],
                                start=True, stop=True,
                            )
                            nc.scalar.activation(att_cm[:, hp, :], patt[:], ACTF.Copy)
                        # Wr + residual -> f3out (+ LN3 stats)
                        scr5 = p1.tile([128, DIM], F32, tag="scr5")
                        for t in range(2):
                            po = ps_mm.tile([128, 512], F32, tag="mm")
                            nc.tensor.matmul(po[:], ones_sb[:], brrow_sb[:], start=True, stop=False)
                            for cc in range(4):
                                nc.tensor.matmul(
                                    po[:],
                                    att_cm[:, cc, t * 128:(t + 1) * 128],
                                    wr_sb[:, cc, :],
                                    start=False, stop=(cc == 3),
                                )
                            f3o = p13.tile([128, DIM], F32, tag="f3o")
                            nc.vector.scalar_tensor_tensor(
                                f3o[:], po[:], 1.0, x3[:, t, :],
                                op0=ALU.mult, op1=ALU.add,
                                accum_out=sum3[:, 2 * s + t: 2 * s + t + 1],
                            )
                            nc.scalar.activation(
                                scr5[:], f3o[:], ACTF.Square,
                                accum_out=sq3[:, 2 * s + t: 2 * s + t + 1],
                            )
                            nc.sync.dma_start(
                                f3o_dram[s, t * 128:(t + 1) * 128, :], f3o[:]
                            )

            # ================= STAGE 2 =================
            with (
                tc.tile_pool(name="wmlp", bufs=1) as wmlp,
                tc.tile_pool(name="s2_sb", bufs=2) as p2,
                tc.tile_pool(name="s2_sb3", bufs=3) as p23,
                tc.tile_pool(name="ps2_tp", bufs=2, space="PSUM") as ps2_tp,
                tc.tile_pool(name="ps2_w1", bufs=2, space="PSUM") as ps2_w1,
                tc.tile_pool(name="ps2_w2", bufs=4, space="PSUM") as ps2_w2,
            ):
                w1_sb = wmlp.tile([128, 4, MLP_DIM], F32R, tag="w1")
                nc.sync.dma_start(w1_sb[:], w1.rearrange("(c p) d -> p c d", p=128))
                w2_sb = wmlp.tile([128, 16, DIM], F32R, tag="w2")
                nc.sync.dma_start(w2_sb[:], w2.rearrange("(c p) d -> p c d", p=128))

                # LN3 stats math for all samples at once
                mean3 = p2.tile([128, 2 * NS], F32, tag="mean3")
                nc.vector.tensor_scalar_mul(mean3[:], sum3[:], 1.0 / DIM)
                m23 = p2.tile([128, 2 * NS], F32, tag="m23")
                nc.vector.tensor_mul(m23[:], mean3[:], mean3[:])
                var3 = p2.tile([128, 2 * NS], F32, tag="var3")
                nc.vector.scalar_tensor_tensor(
                    var3[:], sq3[:], 1.0 / DIM, m23[:], op0=ALU.mult, op1=ALU.subtract
                )
                lnv3 = p2.tile([128, 2 * NS], F32, tag="lnv3")
                nc.scalar.activation(lnv3[:], var3[:], ACTF.Ln, bias=eps_sb[:])
                nc.scalar.activation(s3[:], lnv3[:], ACTF.Exp, scale=-0.5)
                nc.vector.tensor_scalar_mul(negm3[:], mean3[:], -1.0)

                for g in range(NG2):
                    f3o2 = p23.tile([128, 4, DIM], F32, tag="f3o2")
                    for c in range(4):
                        nc.sync.dma_start(
                            f3o2[:, c, :],
                            f3o_dram[2 * g + c // 2, (c % 2) * 128:(c % 2) * 128 + 128, :],
                        )
                    xoh = p2.tile([128, 4, DIM], F32R, tag="xoh")
                    for c in range(4):
                        col = 4 * g + c
                        nc.vector.tensor_scalar(
                            xoh[:, c, :], f3o2[:, c, :],
                            negm3[:, col: col + 1], s3[:, col: col + 1],
                            op0=ALU.add, op1=ALU.mult,
                        )
                    xoc = p2.tile([128, 4, DIM], F32R, tag="xoc")
                    for cc in range(4):
                        ptp = ps2_tp.tile([128, 512], F32R, tag="tp2")
                        for c in range(4):
                            nc.tensor.transpose(
                                ptp[:, c * 128:(c + 1) * 128],
                                xoh[:, c, cc * 128:(cc + 1) * 128],
                                ident_sb[:],
                            )
                        if cc % 2 == 0:
                            nc.vector.tensor_copy(xoc[:, cc, :], ptp[:])
                        else:
                            nc.scalar.activation(xoc[:, cc, :], ptp[:], ACTF.Copy)
                    pf = []
                    for c in range(4):
                        pfc = ps2_w2.tile([128, 512], F32, tag="w2acc")
                        nc.tensor.matmul(pfc[:], ones_sb[:], b2row_sb[:], start=True, stop=False)
                        pf.append(pfc)
                    for hc in range(16):
                        pw1 = ps2_w1.tile([128, 512], F32, tag="w1ps")
                        for kc in range(4):
                            nc.tensor.matmul(
                                pw1[:],
                                w1_sb[:, kc, hc * 128:(hc + 1) * 128],
                                xoc[:, kc, :],
                                start=(kc == 0), stop=(kc == 3),
                            )
                        gt = p23.tile([128, DIM], F32R, tag="gt")
                        nc.scalar.activation(
                            gt[:], pw1[:], ACTF.Gelu, bias=b1_sb[:, hc: hc + 1]
                        )
                        for c in range(4):
                            nc.tensor.matmul(
                                pf[c][:],
                                gt[:, c * 128:(c + 1) * 128],
                                w2_sb[:, hc, :],
                                start=False, stop=(hc == 15),
                                skip_group_check=True,
                            )
                    for c in range(4):
                        outt = p2.tile([128, DIM], F32, tag="outt")
                        nc.vector.tensor_add(outt[:], pf[c][:], f3o2[:, c, :])
                        nc.sync.dma_start(
                            out[2 * g + c // 2, (c % 2) * 128:(c % 2) * 128 + 128, :],
                            outt[:],
                        )

    nc.compile()
    return nc


def _get_module(n_samples):
    if n_samples not in _BUILD_CACHE:
        _BUILD_CACHE[n_samples] = _build(n_samples)
    return _BUILD_CACHE[n_samples]


def kernel(**inputs) -> np.ndarray:
    from concourse.bass_utils import run_bass_kernel_spmd

    consts = _host_prep(inputs)
    f3 = np.ascontiguousarray(np.asarray(inputs["f3"], dtype=np.float32))
    f4 = np.ascontiguousarray(np.asarray(inputs["f4"], dtype=np.float32))

    nc = _get_module(BSH)
    in_maps = []
    for c in range(N_CORES):
        m = dict(consts)
        m["f3"] = np.ascontiguousarray(f3[c * BSH:(c + 1) * BSH])
        m["f4"] = np.ascontiguousarray(f4[c * BSH:(c + 1) * BSH])
        in_maps.append(m)
    res = run_bass_kernel_spmd(nc, in_maps, core_ids=list(range(N_CORES)))
    return np.concatenate([res.results[c]["out"] for c in range(N_CORES)], axis=0)
